# revision 13
# baseline (speedup 1.0000x reference)
"""Trainium2 Bass kernel for nn_BatchedCauchyKernel.

Computes, for x[N,D], y[M,D], sample_x[N,S], sample_y[M,S], scale[S]:
    d[i,j]   = |x_i|^2 + |y_j|^2 - 2 x_i.y_j
    sx_i     = clip(softplus(sample_x_i . scale), 1e-10, 1e4)
    sy_j     = clip(softplus(sample_y_j . scale), 1e-10, 1e4)
    res      = 1 / (1 + d / sqrt(sx_i * sy_j))
    out      = res * sigmoid(phi * (res - clip(cutoff, 0, 1000)))

Strategy (8 NeuronCores, row-parallel over N):
  Each core handles NS = N/8 rows of x and the full y.  Let
  rsx_i = 1/sqrt(sx_i), rsy_j = 1/sqrt(sy_j).  Then
    1 + d*rsx*rsy = 1 + a_i*rsy_j + rsx_i*b_j + sum_k xp[i,k]*yp[j,k]
  with a = |x|^2 * rsx, b = |y|^2 * rsy, xp = -2*x*rsx, yp = y*rsy.
  This is a single K=(D+7) matmul in bf16 (extension rows carry the
  rank-1 terms, split hi/lo in bf16 for precision, plus a row of ones),
  accumulated in PSUM.  Epilogue per [128,512] tile:
    res = reciprocal_approx_fast(psum)          (DVE)
    mask = sigmoid(phi*res - phi*cutoff)        (ACT)
    out = res * mask                            (DVE)
"""

import os
import sys

sys.path.insert(0, "/opt/trn_rl_repo")

import numpy as np

N, M, D, S = 8192, 4096, 512, 16
CORES = 8
NS = N // CORES  # 1024 rows of x per core
PO = NS // 128  # 8 i-tiles per core
JT = M // 512  # 8 j-tiles
KT = D // 128  # 4 k-tiles
NEXT = 7  # number of extension contraction rows

SOFTPLUS_MIN = 1e-10
SOFTPLUS_MAX = 10000.0

_CACHE = {}


def _build(phi_val: float, cutoff_val: float):
    import concourse.mybir as mybir
    import concourse.tile as tile
    from concourse import bacc
    from concourse.masks import make_identity

    dt = mybir.dt
    AF = mybir.ActivationFunctionType
    OP = mybir.AluOpType

    nc = bacc.Bacc("TRN2", target_bir_lowering=False)

    x_d = nc.dram_tensor("x_shard", [NS, D], dt.float32, kind="ExternalInput")
    y_d = nc.dram_tensor("y_full", [M, D], dt.float32, kind="ExternalInput")
    sx_d = nc.dram_tensor("sample_x_shard", [NS, S], dt.float32, kind="ExternalInput")
    sy_d = nc.dram_tensor("sample_y_full", [M, S], dt.float32, kind="ExternalInput")
    sc_d = nc.dram_tensor("scale_full", [1, S], dt.float32, kind="ExternalInput")
    out_d = nc.dram_tensor("out_shard", [NS, M], dt.float32, kind="ExternalOutput")

    # DRAM views:
    #  x rows i = po*128 + pi  (pi = partition)
    x_v = x_d.rearrange("(po pi) k -> pi po k", pi=128)  # [128, PO, D]
    sx_v = sx_d.rearrange("(po pi) s -> pi po s", pi=128)  # [128, PO, S]
    out_v = out_d.rearrange("(po pi) j -> pi po j", pi=128)  # [128, PO, M]
    #  y rows j = a*32 + b  (a = partition) -> contiguous [M] vectors on DRAM
    y_v = y_d.rearrange("(a b) k -> a b k", a=128)  # [128, 32, D]
    sy_v = sy_d.rearrange("(a b) s -> a b s", a=128)  # [128, 32, S]

    with tile.TileContext(nc) as tc:
        with (
            tc.tile_pool(name="persist", bufs=1) as persist,
            tc.tile_pool(name="prep", bufs=2) as prep,
            tc.tile_pool(name="dram", bufs=1, space="DRAM") as dram,
            tc.tile_pool(name="psum_t", bufs=1, space="PSUM") as psum_t,
            tc.tile_pool(name="psum", bufs=7, space="PSUM") as psum_p,
            tc.tile_pool(name="main", bufs=3) as main,
        ):
            # ---------------- small-vector prep (x side) ----------------
            sc_sb0 = persist.tile([1, S], dt.float32)
            nc.sync.dma_start(sc_sb0[:], sc_d[:, :])
            sc_sb = persist.tile([1, S], dt.float32)
            nc.vector.tensor_copy(sc_sb[:], sc_sb0[:])
            ones_row = persist.tile([1, 128], dt.float32)
            nc.vector.memset(ones_row[:], 1.0)
            sc_ps = psum_t.tile([128, S], dt.float32, tag="small_ps")
            nc.tensor.matmul(sc_ps[:], lhsT=ones_row[:], rhs=sc_sb[:], start=True, stop=True)
            scale_rep = persist.tile([128, S], dt.float32)
            nc.vector.tensor_copy(scale_rep[:], sc_ps[:])

            sxs = prep.tile([128, PO, S], dt.float32, tag="sxs")
            nc.sync.dma_start(sxs[:], sx_v)
            tmp_x = prep.tile([128, PO, S], dt.float32, tag="tmp_x")
            nc.vector.tensor_tensor(
                tmp_x[:], sxs[:],
                scale_rep[:, None, :].to_broadcast((128, PO, S)), OP.mult,
            )
            sxr = persist.tile([128, PO], dt.float32)  # sample_x . scale
            nc.vector.tensor_reduce(
                sxr[:, :, None], tmp_x[:], mybir.AxisListType.X, OP.add
            )
            # v = clip(softplus(sxr)); rsx = v**-0.5   (natural_log_exp set)
            vx = persist.tile([128, PO], dt.float32)
            nc.scalar.activation(vx[:], sxr[:], AF.Exp)
            nc.scalar.activation(vx[:], vx[:], AF.Ln, bias=1.0)
            nc.vector.tensor_scalar(
                vx[:], vx[:], SOFTPLUS_MAX, SOFTPLUS_MIN, OP.min, OP.max
            )
            rsx = persist.tile([128, PO], dt.float32)
            nc.scalar.activation(rsx[:], vx[:], AF.Ln)
            nc.scalar.activation(rsx[:], rsx[:], AF.Exp, scale=-0.5)

            # ---------------- x tiles: sq_x, xp ----------------
            x_sb = prep.tile([128, PO, D], dt.float32, tag="x_sb")
            nc.sync.dma_start(x_sb[:], x_v)
            sqx = persist.tile([128, PO], dt.float32)
            sq_scratch = prep.tile([128, D], dt.float32, tag="sq_scratch")
            for po in range(PO):
                nc.scalar.activation(
                    sq_scratch[:], x_sb[:, po, :], AF.Square,
                    accum_out=sqx[:, po, None],
                )
            a_x = persist.tile([128, PO], dt.float32)
            nc.vector.tensor_tensor(a_x[:], sqx[:], rsx[:], OP.mult)

            # xp = -2 * x * rsx  (bf16)
            rsx_n2 = persist.tile([128, PO], dt.float32)
            nc.vector.tensor_scalar_mul(rsx_n2[:], rsx[:], -2.0)
            xp_sb = prep.tile([128, PO, D], dt.bfloat16, tag="xp_sb")
            for po in range(PO):
                nc.vector.tensor_scalar_mul(
                    xp_sb[:, po, :], x_sb[:, po, :], rsx_n2[:, po, None]
                )
            xp_dram = dram.tile([NS, D], dt.bfloat16)
            nc.sync.dma_start(
                xp_dram.rearrange("(po pi) k -> pi po k", pi=128), xp_sb[:]
            )

            # hi/lo bf16 split of a_x and rsx, packed for transposition.
            # ext row order r (lhsT value x rhs value):
            #   0: a_hi  * rsy_hi     1: a_hi * rsy_lo   2: a_lo * rsy_hi
            #   3: rsx_hi* b_hi       4: rsx_hi * b_lo   5: rsx_lo * b_hi
            #   6: 1 * 1
            def hi_lo(vec, tag):
                hi_b = prep.tile([128, PO], dt.bfloat16, tag=f"{tag}_hb")
                nc.vector.tensor_copy(hi_b[:], vec[:])
                hi_f = prep.tile([128, PO], dt.float32, tag=f"{tag}_hf")
                nc.vector.tensor_copy(hi_f[:], hi_b[:])
                lo_f = prep.tile([128, PO], dt.float32, tag=f"{tag}_lf")
                nc.vector.tensor_tensor(lo_f[:], vec[:], hi_f[:], OP.subtract)
                return hi_f, lo_f

            a_hi, a_lo = hi_lo(a_x, "a")
            r_hi, r_lo = hi_lo(rsx, "r")
            ext_pack = prep.tile([128, PO, NEXT], dt.float32, tag="ext_pack")
            for r, src in enumerate([a_hi, a_hi, a_lo, r_hi, r_hi, r_lo, None]):
                if src is None:
                    nc.vector.memset(ext_pack[:, :, r], 1.0)
                else:
                    nc.vector.tensor_copy(ext_pack[:, :, r], src[:])
            ident0 = persist.tile([128, 128], dt.float32)
            make_identity(nc, ident0[:])
            ident = persist.tile([128, 128], dt.float32)
            nc.vector.tensor_copy(ident[:], ident0[:])
            lhsT_ext = []
            for po in range(PO):
                extT_full = psum_t.tile([128, 128], dt.float32, tag="small_ps")
                extT_ps = extT_full[:NEXT]
                nc.tensor.transpose(extT_ps[:], ext_pack[:, po, :], ident[:])
                t = persist.tile([NEXT, 128], dt.bfloat16, tag=f"lhsT_ext{po}")
                nc.vector.tensor_copy(t[:], extT_ps[:])
                lhsT_ext.append(t)

            # ---------------- y side small vectors ----------------
            sys_ = prep.tile([128, 32, S], dt.float32, tag="sys")
            nc.sync.dma_start(sys_[:], sy_v)
            tmp_y = prep.tile([128, 32, S], dt.float32, tag="tmp_y")
            nc.vector.tensor_tensor(
                tmp_y[:], sys_[:],
                scale_rep[:, None, :].to_broadcast((128, 32, S)), OP.mult,
            )
            syr = persist.tile([128, 32], dt.float32)
            nc.vector.tensor_reduce(
                syr[:, :, None], tmp_y[:], mybir.AxisListType.X, OP.add
            )
            vy = persist.tile([128, 32], dt.float32)
            nc.scalar.activation(vy[:], syr[:], AF.Exp)
            nc.scalar.activation(vy[:], vy[:], AF.Ln, bias=1.0)
            nc.vector.tensor_scalar(
                vy[:], vy[:], SOFTPLUS_MAX, SOFTPLUS_MIN, OP.min, OP.max
            )
            rsy = persist.tile([128, 32], dt.float32)
            nc.scalar.activation(rsy[:], vy[:], AF.Ln)
            nc.scalar.activation(rsy[:], rsy[:], AF.Exp, scale=-0.5)

            # ---------------- y tiles: sq_y, yp ----------------
            sqy = persist.tile([128, 32], dt.float32)
            yp_dram = dram.tile([M, D], dt.bfloat16)
            YG = 8  # b-chunk size
            for g in range(32 // YG):
                y_sb = prep.tile([128, YG, D], dt.float32, tag="y_sb")
                nc.sync.dma_start(y_sb[:], y_v[:, g * YG:(g + 1) * YG, :])
                yp_sb = prep.tile([128, YG, D], dt.bfloat16, tag="yp_sb")
                for b in range(YG):
                    bb = g * YG + b
                    nc.scalar.activation(
                        sq_scratch[:], y_sb[:, b, :], AF.Square,
                        accum_out=sqy[:, bb, None],
                    )
                    nc.vector.tensor_scalar_mul(
                        yp_sb[:, b, :], y_sb[:, b, :], rsy[:, bb, None]
                    )
                nc.sync.dma_start(
                    yp_dram.rearrange("(a b) k -> a b k", a=128)[
                        :, g * YG:(g + 1) * YG, :
                    ],
                    yp_sb[:],
                )

            b_y = persist.tile([128, 32], dt.float32)
            nc.vector.tensor_tensor(b_y[:], sqy[:], rsy[:], OP.mult)

            # hi/lo of rsy and b_y -> DRAM (contiguous [M] bf16), then load
            # back as [1, M] rows to build the rhs extension block.
            def hi_lo_y(vec, tag):
                hi_b = prep.tile([128, 32], dt.bfloat16, tag=f"{tag}_yhb")
                nc.vector.tensor_copy(hi_b[:], vec[:])
                hi_f = prep.tile([128, 32], dt.float32, tag=f"{tag}_yhf")
                nc.vector.tensor_copy(hi_f[:], hi_b[:])
                lo_f = prep.tile([128, 32], dt.float32, tag=f"{tag}_ylf")
                nc.vector.tensor_tensor(lo_f[:], vec[:], hi_f[:], OP.subtract)
                lo_b = prep.tile([128, 32], dt.bfloat16, tag=f"{tag}_ylb")
                nc.vector.tensor_copy(lo_b[:], lo_f[:])
                return hi_b, lo_b

            rsy_hi, rsy_lo = hi_lo_y(rsy, "rsy")
            by_hi, by_lo = hi_lo_y(b_y, "by")
            vec_dram = dram.tile([4, M], dt.bfloat16)
            for r, src in enumerate([rsy_hi, rsy_lo, by_hi, by_lo]):
                nc.sync.dma_start(
                    vec_dram[r, :].rearrange("(a b) -> a b", a=128), src[:]
                )
            rhs_ext = persist.tile([NEXT, M], dt.bfloat16)
            nc.vector.memset(rhs_ext[:], 1.0)  # row 6 stays all-ones
            for r, v in enumerate([0, 1, 0, 2, 3, 2]):  # see ext row order
                nc.sync.dma_start(rhs_ext[r:r + 1, :], vec_dram[v:v + 1, :])

            # ---------------- transposed loads ----------------
            xpT = persist.tile([128, KT, NS], dt.bfloat16)
            ypT = persist.tile([128, KT, M], dt.bfloat16)
            for kt in range(KT):
                nc.sync.dma_start_transpose(
                    xpT[:, kt, :], xp_dram[:, kt * 128:(kt + 1) * 128]
                )
                nc.sync.dma_start_transpose(
                    ypT[:, kt, :], yp_dram[:, kt * 128:(kt + 1) * 128]
                )

            # ---------------- main loop ----------------
            sig_scale = phi_val
            sig_bias = persist.tile([128, 1], dt.float32)
            nc.vector.memset(sig_bias[:], -phi_val * cutoff_val)
            for po in range(PO):
                for jt in range(JT):
                    ps = psum_p.tile([128, 512], dt.float32, tag="mm")
                    for kt in range(KT):
                        nc.tensor.matmul(
                            ps[:],
                            lhsT=xpT[:, kt, po * 128:(po + 1) * 128],
                            rhs=ypT[:, kt, jt * 512:(jt + 1) * 512],
                            start=(kt == 0),
                            stop=False,
                        )
                    nc.tensor.matmul(
                        ps[:],
                        lhsT=lhsT_ext[po][:],
                        rhs=rhs_ext[:, jt * 512:(jt + 1) * 512],
                        start=False,
                        stop=True,
                    )
                    res = main.tile([128, 512], dt.float32, tag="res")
                    nc.vector.reciprocal_approx_fast(res[:], ps[:])
                    mask = main.tile([128, 512], dt.float32, tag="mask")
                    nc.scalar.activation(
                        mask[:], res[:], AF.Sigmoid,
                        bias=sig_bias[:], scale=sig_scale,
                    )
                    ot = main.tile([128, 512], dt.float32, tag="ot")
                    nc.vector.tensor_tensor(ot[:], res[:], mask[:], OP.mult)
                    nc.sync.dma_start(
                        out_v[:, po, jt * 512:(jt + 1) * 512], ot[:]
                    )

    nc.compile()
    return nc


def kernel(x, y, sample_x, sample_y, scale, cutoff, phi):
    from concourse.bass_utils import run_bass_kernel_spmd

    phi_val = float(np.asarray(phi).reshape(-1)[0])
    cutoff_val = float(np.clip(np.asarray(cutoff).reshape(-1)[0], 0.0, 1000.0))

    key = (phi_val, cutoff_val)
    if key not in _CACHE:
        _CACHE[key] = _build(phi_val, cutoff_val)
    nc = _CACHE[key]

    x = np.ascontiguousarray(np.asarray(x, dtype=np.float32))
    y = np.ascontiguousarray(np.asarray(y, dtype=np.float32))
    sample_x = np.ascontiguousarray(np.asarray(sample_x, dtype=np.float32))
    sample_y = np.ascontiguousarray(np.asarray(sample_y, dtype=np.float32))
    scale = np.ascontiguousarray(np.asarray(scale, dtype=np.float32)).reshape(1, S)

    in_maps = []
    for c in range(CORES):
        in_maps.append(
            {
                "x_shard": x[c * NS:(c + 1) * NS],
                "y_full": y,
                "sample_x_shard": sample_x[c * NS:(c + 1) * NS],
                "sample_y_full": sample_y,
                "scale_full": scale,
            }
        )

    trace = bool(int(os.environ.get("KERNEL_TRACE", "0")))
    r = run_bass_kernel_spmd(nc, in_maps, core_ids=list(range(CORES)), trace=trace)
    kernel.last_results = r
    out = np.concatenate([m["out_shard"] for m in r.results], axis=0)
    return out


if __name__ == "__main__":
    rng = np.random.default_rng(0)
    ins = {
        "x": rng.standard_normal((N, D), dtype=np.float32),
        "y": rng.standard_normal((M, D), dtype=np.float32),
        "sample_x": rng.random((N, S), dtype=np.float32),
        "sample_y": rng.random((M, S), dtype=np.float32),
        "scale": rng.random((S,), dtype=np.float32),
        "cutoff": np.full((1,), 0.1, dtype=np.float32),
        "phi": np.ones((1,), dtype=np.float32),
    }
    o = kernel(**ins)
    print(o.shape, o.dtype, o[:2, :4])
